# revision 1
# baseline (speedup 1.0000x reference)
"""Trainium2 kernel for nn_MiddleHeadLayer: 2-layer tanh MLP + row-dot + sigmoid.

    inner = tanh(batch @ W1.T + b1)        batch [N, 1024], W1 [4096, 1024]
    wx    = tanh(inner @ W2.T + b2)        W2 [1024, 4096]
    out   = sigmoid(sum(wx * batch, -1))   [N]

Data-parallel over 8 NeuronCores: each core handles N/8 = 2048 rows;
weights replicated, resident in SBUF as fp16 (f32 weights do not fit in
24MB SBUF; fp16 matmuls run at full PE rate and keep absmax error ~3e-3).

Per-core dataflow, in blocks of R=256 rows:
  phase 1: innerT[dff, rows] = tanh(W1T.T @ batchT + b1) — stationary W1T
           chunks [128,128], moving batchT [128, R], fp16 in / f32 PSUM,
           ACT applies the per-partition (d_ff) bias and writes fp16.
  phase 2: wx[rows, dmodel] = tanh(innerT.T @ W2T + b2) — stationary innerT
           chunks, moving W2T [128, 512]. b2 (free-dim bias) is folded in
           as a rank-1 ones x b2 matmul into the same PSUM group.
  dot:     z[rows] = sum(wx * batch_f32) via fused DVE tensor_tensor_reduce
           along the free dim; sigmoid once at the end on all z columns.
"""

from contextlib import ExitStack

import numpy as np
import orjson

import concourse.bass as bass
import concourse.tile as tile
from concourse import mybir
from concourse import bass_utils

D_MODEL = 1024
D_FF = 4096
N_TOTAL = 16384
N_CORES = 8
NC_ROWS = N_TOTAL // N_CORES          # 2048 rows per core
R = 256                               # row-block size
N_BLOCKS = NC_ROWS // R               # 8
K1 = D_MODEL // 128                   # 8 contraction chunks for matmul1
M1 = D_FF // 128                      # 32 d_ff chunks
RG = R // 128                         # row groups per block
NH = D_MODEL // 512                   # d_model halves for phase 2
F16 = mybir.dt.float16
F32 = mybir.dt.float32


# ---------------------------------------------------------------------------
# This walrus build rejects >2 sem waits on a single instruction, while Tile's
# wait assignment freely attaches more (e.g. the exit drain gets one wait per
# outstanding logical proc). Legalize at the BIR-JSON level: hoist excess
# waits onto EventSemaphore instructions inserted directly before the
# offending instruction on the same engine stream (identical semantics).
MAX_WAITS = 1


def _legalize_sync_waits(bir: dict) -> dict:
    ctr = 0
    for fn in bir.get("functions", []):
        for blk in fn.get("blocks", []):
            insts = blk.get("instructions")
            if not insts:
                continue
            out = []
            changed = False
            for inst in insts:
                si = inst.get("sync_info")
                ow = (si or {}).get("on_wait") or []
                limit = 2 if inst.get("opcode") == "EventSemaphore" else MAX_WAITS
                if len(ow) > limit:
                    changed = True
                    excess, keep = ow[:-limit], ow[-limit:]
                    for i in range(0, len(excess), MAX_WAITS):
                        ctr += 1
                        out.append({
                            "debug": inst.get("debug"),
                            "engine": inst["engine"],
                            "ins": [],
                            "outs": [],
                            "name": f"legalwait-{ctr}",
                            "opcode": "EventSemaphore",
                            "sync_info": {
                                "on_update": [],
                                "on_wait": excess[i:i + MAX_WAITS],
                            },
                        })
                    si["on_wait"] = keep
                out.append(inst)
            if changed:
                blk["instructions"] = out
    return bir


_orig_to_json_bytes = bass.Bass.to_json_bytes


def _patched_to_json_bytes(self) -> bytes:
    return orjson.dumps(_legalize_sync_waits(orjson.loads(_orig_to_json_bytes(self))))


bass.Bass.to_json_bytes = _patched_to_json_bytes


def build_bass(n_blocks=N_BLOCKS):
    nc = bass.Bass("TRN2", target_bir_lowering=False, debug=False)

    w1t_d = nc.dram_tensor("w1t", [D_MODEL, D_FF], F16, kind="ExternalInput")
    w2t_d = nc.dram_tensor("w2t", [D_FF, D_MODEL], F16, kind="ExternalInput")
    b1_d = nc.dram_tensor("b1c", [128, M1], F32, kind="ExternalInput")
    b2_d = nc.dram_tensor("b2c", [1, D_MODEL], F16, kind="ExternalInput")
    ones_d = nc.dram_tensor("ones", [1, 128], F16, kind="ExternalInput")
    bt_d = nc.dram_tensor("batcht", [D_MODEL, NC_ROWS], F16, kind="ExternalInput")
    b_d = nc.dram_tensor("batch", [NC_ROWS, D_MODEL], F32, kind="ExternalInput")
    out_d = nc.dram_tensor("out", [NC_ROWS, 1], F32, kind="ExternalOutput")

    n_groups = n_blocks * RG
    W1CB = 4                       # w1t column blocks (of 1024 d_ff each)

    with tile.TileContext(nc) as tc, ExitStack() as ctx:
        wpool = ctx.enter_context(tc.tile_pool(name="weights", bufs=1))
        btpool = ctx.enter_context(tc.tile_pool(name="batchT", bufs=16))
        ipool = ctx.enter_context(tc.tile_pool(name="innerT", bufs=36))
        bfpool = ctx.enter_context(tc.tile_pool(name="batchf", bufs=4))
        wxpool = ctx.enter_context(tc.tile_pool(name="wx", bufs=4))
        spool = ctx.enter_context(tc.tile_pool(name="scratch", bufs=2))
        zpool = ctx.enter_context(tc.tile_pool(name="z", bufs=1))
        psum1 = ctx.enter_context(tc.tile_pool(name="psum1", bufs=3, space="PSUM"))
        psum2 = ctx.enter_context(tc.tile_pool(name="psum2", bufs=4, space="PSUM"))

        # DMA emission order = queue order: block-0 activations and the first
        # w1t column block go first so PE can start ~15us in; the remaining
        # weight bulk streams behind them.
        ones = wpool.tile([1, 128], F16, tag="ones")
        nc.sync.dma_start(ones[:], ones_d.ap()[:])

        bt0 = []
        for k in range(K1):
            t = btpool.tile([128, R], F16, tag="bt")
            nc.sync.dma_start(t[:], bt_d.ap()[k * 128:(k + 1) * 128, 0:R])
            bt0.append(t)

        CBW = D_FF // W1CB
        w1t = [[None] * W1CB for _ in range(K1)]
        for k in range(K1):
            t = wpool.tile([128, CBW], F16, tag=f"w1t{k}c0")
            nc.sync.dma_start(t[:], w1t_d.ap()[k * 128:(k + 1) * 128, 0:CBW])
            w1t[k][0] = t

        b1t = wpool.tile([128, M1], F32, tag="b1t")
        nc.sync.dma_start(b1t[:], b1_d.ap()[:])
        b2t = wpool.tile([1, D_MODEL], F16, tag="b2t")
        nc.sync.dma_start(b2t[:], b2_d.ap()[:])

        # rest of W1T column blocks
        for cb in range(1, W1CB):
            for k in range(K1):
                t = wpool.tile([128, CBW], F16, tag=f"w1t{k}c{cb}")
                nc.sync.dma_start(
                    t[:], w1t_d.ap()[k * 128:(k + 1) * 128, cb * CBW:(cb + 1) * CBW]
                )
                w1t[k][cb] = t
        # W2T chunks (first needed ~45us in, at phase 2 of block 0)
        w2t = []
        for m in range(M1):
            t = wpool.tile([128, D_MODEL], F16, tag=f"w2t{m}")
            nc.sync.dma_start(t[:], w2t_d.ap()[m * 128:(m + 1) * 128, :])
            w2t.append(t)

        z_all = zpool.tile([128, n_groups], F32)
        sig = zpool.tile([128, n_groups], F32, tag="sig")

        for b in range(n_blocks):
            # batchT chunks for this row block
            if b == 0:
                bt = bt0
            else:
                bt = []
                for k in range(K1):
                    t = btpool.tile([128, R], F16, tag="bt")
                    nc.sync.dma_start(
                        t[:], bt_d.ap()[k * 128:(k + 1) * 128, b * R:(b + 1) * R]
                    )
                    bt.append(t)

            # phase 1: innerT chunks [128 dff, R rows]
            it = []
            for m in range(M1):
                cb, mo = divmod(m, CBW // 128)
                ps = psum1.tile([128, R], F32)
                for k in range(K1):
                    nc.tensor.matmul(
                        ps[:],
                        w1t[k][cb][:, mo * 128:(mo + 1) * 128],
                        bt[k][:],
                        start=(k == 0),
                        stop=(k == K1 - 1),
                    )
                t = ipool.tile([128, R], F16, tag="it")
                nc.scalar.activation(
                    t[:], ps[:], mybir.ActivationFunctionType.Tanh,
                    bias=b1t[:, m:m + 1],
                )
                it.append(t)

            # phase 2 + row-dot per 128-row group
            for rg in range(RG):
                g = b * RG + rg
                bf = bfpool.tile([128, D_MODEL], F32, tag="bf")
                nc.sync.dma_start(
                    bf[:], b_d.ap()[g * 128:(g + 1) * 128, :]
                )
                wx = wxpool.tile([128, D_MODEL], F32, tag="wx")
                for h in range(NH):
                    ps2 = psum2.tile([128, 512], F32)
                    for m in range(M1):
                        nc.tensor.matmul(
                            ps2[:],
                            it[m][:, rg * 128:(rg + 1) * 128],
                            w2t[m][:, h * 512:(h + 1) * 512],
                            start=(m == 0),
                            stop=False,
                        )
                    # b2 (free-dim bias) as a rank-1 ones x b2 accumulate,
                    # last so the group opener is a regular K=128 matmul
                    nc.tensor.matmul(
                        ps2[:],
                        ones[:],
                        b2t[:, h * 512:(h + 1) * 512],
                        start=False,
                        stop=True,
                    )
                    nc.scalar.activation(
                        wx[:, h * 512:(h + 1) * 512], ps2[:],
                        mybir.ActivationFunctionType.Tanh,
                    )
                # z[g] = sum(wx * batch) along d_model, fused mult+reduce on DVE
                scratch = spool.tile([128, D_MODEL], F32, tag="scr")
                nc.vector.scalar_tensor_tensor(
                    out=scratch[:],
                    in0=wx[:],
                    scalar=1.0,
                    in1=bf[:],
                    op0=mybir.AluOpType.mult,
                    op1=mybir.AluOpType.mult,
                    accum_out=z_all[:, g:g + 1],
                )
                nc.scalar.activation(
                    sig[:, g:g + 1], z_all[:, g:g + 1],
                    mybir.ActivationFunctionType.Sigmoid,
                )
                nc.sync.dma_start(
                    out_d.ap()[g * 128:(g + 1) * 128, :], sig[:, g:g + 1]
                )

    return nc


_CACHED = {}


def _get_nc(n_blocks=N_BLOCKS):
    if n_blocks not in _CACHED:
        _CACHED[n_blocks] = build_bass(n_blocks)
    return _CACHED[n_blocks]


def _prep_in_maps(batch, W1, b1, W2, b2):
    batch = np.ascontiguousarray(batch, dtype=np.float32)
    w1t = np.ascontiguousarray(W1.T, dtype=np.float16)      # [1024, 4096]
    w2t = np.ascontiguousarray(W2.T, dtype=np.float16)      # [4096, 1024]
    # b1 as [128, 32]: column m holds b1[m*128:(m+1)*128] (per-partition bias)
    b1c = np.ascontiguousarray(
        np.asarray(b1, dtype=np.float32).reshape(M1, 128).T
    )
    b2c = np.ascontiguousarray(b2, dtype=np.float16).reshape(1, D_MODEL)
    ones = np.ones((1, 128), dtype=np.float16)
    batcht = np.ascontiguousarray(batch.T.astype(np.float16))  # [1024, 16384]

    in_maps = []
    for c in range(N_CORES):
        r0, r1 = c * NC_ROWS, (c + 1) * NC_ROWS
        in_maps.append({
            "w1t": w1t,
            "w2t": w2t,
            "b1c": b1c,
            "b2c": b2c,
            "ones": ones,
            "batcht": np.ascontiguousarray(batcht[:, r0:r1]),
            "batch": np.ascontiguousarray(batch[r0:r1]),
        })
    return in_maps


def kernel(batch, W1, b1, W2, b2, _trace=False, _trace_kwargs=None):
    in_maps = _prep_in_maps(batch, W1, b1, W2, b2)
    nc = _get_nc()
    res = bass_utils.run_bass_kernel_spmd(
        nc, in_maps, core_ids=list(range(N_CORES)),
        trace=_trace, **(_trace_kwargs or {}),
    )
    out = np.concatenate([res.results[c]["out"][:, 0] for c in range(N_CORES)])
    if _trace:
        return out, res
    return out



# revision 2
# speedup vs baseline: 1.0264x; 1.0264x over previous
"""Trainium2 kernel for nn_MiddleHeadLayer: 2-layer tanh MLP + row-dot + sigmoid.

    inner = tanh(batch @ W1.T + b1)        batch [N, 1024], W1 [4096, 1024]
    wx    = tanh(inner @ W2.T + b2)        W2 [1024, 4096]
    out   = sigmoid(sum(wx * batch, -1))   [N]

Data-parallel over 8 NeuronCores: each core handles N/8 = 2048 rows;
weights replicated, resident in SBUF as fp16 (f32 weights do not fit in
24MB SBUF; fp16 matmuls run at full PE rate and keep absmax error ~3e-3).

Per-core dataflow, in blocks of R=512 rows:
  phase 1: innerT[dff, rows] = tanh(W1T.T @ batchT + b1) — stationary W1T
           chunks [128,128], moving batchT [128, 512], fp16 in / f32 PSUM,
           ACT applies the per-partition (d_ff) bias and writes fp16.
  phase 2: wxT[dmodel, rows] = tanh(W2 @ inner.T + b2) — stationary W2T
           chunks [128 dff, 128 dmodel], moving innerT [128, 512]. The
           output partition dim is d_model, so b2 is a plain per-partition
           ACT bias (no extra rank-1 bias matmul needed).
  dot:     acc[dm_chunk, rows] = sum_h wxT_h * batchT_h on DVE (the same
           fp16 batchT tiles phase 1 streams), then a [128,1] ones fp32
           matmul folds the 128 partitions into s[1, rows]; sigmoid on ACT;
           one contiguous 2KB output DMA per block.

DMA order is chosen so the first matmul can start ~4us in: batchT block 0
and the first W1T column chunks go first in small pieces, then W1/W2
stream in exactly the order the PE consumes them.
"""

from contextlib import ExitStack

import numpy as np
import orjson

import concourse.bass as bass
import concourse.tile as tile
from concourse import mybir
from concourse import bass_utils

D_MODEL = 1024
D_FF = 4096
N_TOTAL = 16384
N_CORES = 8
NC_ROWS = N_TOTAL // N_CORES          # 2048 rows per core
R = 512                               # row-block size
N_BLOCKS = NC_ROWS // R               # 4
K1 = D_MODEL // 128                   # 8 contraction chunks for matmul1
M1 = D_FF // 128                      # 32 d_ff chunks
MB = 4                                # m-chunks per w1 column tile
NMB = M1 // MB                        # 8 w1 column tiles per k
H = D_MODEL // 128                    # 8 d_model chunks in phase 2
MG = 4                                # m-chunks per w2 tile
NMG = M1 // MG                        # 8 w2 tiles per h
F16 = mybir.dt.float16
F32 = mybir.dt.float32


# ---------------------------------------------------------------------------
# This walrus build rejects >2 sem waits on a single instruction, while Tile's
# wait assignment freely attaches more (e.g. the exit drain gets one wait per
# outstanding logical proc). Legalize at the BIR-JSON level: hoist excess
# waits onto EventSemaphore instructions inserted directly before the
# offending instruction on the same engine stream (identical semantics).
MAX_WAITS = 1
ESEM_WAITS = 2  # EventSemaphore instructions themselves may carry 2 waits


def _legalize_sync_waits(bir: dict) -> dict:
    ctr = 0
    for fn in bir.get("functions", []):
        for blk in fn.get("blocks", []):
            insts = blk.get("instructions")
            if not insts:
                continue
            out = []
            changed = False
            for inst in insts:
                si = inst.get("sync_info")
                ow = (si or {}).get("on_wait") or []
                limit = ESEM_WAITS if inst.get("opcode") == "EventSemaphore" else MAX_WAITS
                if len(ow) > limit:
                    changed = True
                    excess, keep = ow[:-limit], ow[-limit:]
                    for i in range(0, len(excess), ESEM_WAITS):
                        ctr += 1
                        out.append({
                            "debug": inst.get("debug"),
                            "engine": inst["engine"],
                            "ins": [],
                            "outs": [],
                            "name": f"legalwait-{ctr}",
                            "opcode": "EventSemaphore",
                            "sync_info": {
                                "on_update": [],
                                "on_wait": excess[i:i + ESEM_WAITS],
                            },
                        })
                    si["on_wait"] = keep
                out.append(inst)
            if changed:
                blk["instructions"] = out
    return bir


_orig_to_json_bytes = bass.Bass.to_json_bytes


def _patched_to_json_bytes(self) -> bytes:
    return orjson.dumps(_legalize_sync_waits(orjson.loads(_orig_to_json_bytes(self))))


bass.Bass.to_json_bytes = _patched_to_json_bytes


def build_bass(n_blocks=N_BLOCKS):
    nc = bass.Bass("TRN2", target_bir_lowering=False, debug=False)

    w1t_d = nc.dram_tensor("w1t", [D_MODEL, D_FF], F16, kind="ExternalInput")
    # w2p: W2.T pre-packed host-side as [H, NMG, 128, MG*128] so each
    # (h, mg) stationary group is one contiguous 128KB tile.
    w2p_d = nc.dram_tensor("w2p", [H * NMG * 128, MG * 128], F16, kind="ExternalInput")
    b1_d = nc.dram_tensor("b1c", [128, M1], F32, kind="ExternalInput")
    b2_d = nc.dram_tensor("b2c", [128, H], F32, kind="ExternalInput")
    ones_d = nc.dram_tensor("ones", [128, 1], F32, kind="ExternalInput")
    bt_d = nc.dram_tensor("batcht", [D_MODEL, NC_ROWS], F16, kind="ExternalInput")
    out_d = nc.dram_tensor("out", [1, NC_ROWS], F32, kind="ExternalOutput")

    with tile.TileContext(nc) as tc, ExitStack() as ctx:
        wpool = ctx.enter_context(tc.tile_pool(name="weights", bufs=1))
        btpool = ctx.enter_context(tc.tile_pool(name="batchT", bufs=16))
        ipool = ctx.enter_context(tc.tile_pool(name="innerT", bufs=32))
        vpool = ctx.enter_context(tc.tile_pool(name="dot", bufs=2))
        psum1 = ctx.enter_context(tc.tile_pool(name="psum1", bufs=2, space="PSUM"))
        psum2 = ctx.enter_context(tc.tile_pool(name="psum2", bufs=2, space="PSUM"))
        psumS = ctx.enter_context(tc.tile_pool(name="psumS", bufs=2, space="PSUM"))

        # --- DMA emission order = queue order -----------------------------
        # tiny constants first
        ones = wpool.tile([128, 1], F32, tag="ones")
        nc.sync.dma_start(ones[:], ones_d.ap()[:])
        b1t = wpool.tile([128, M1], F32, tag="b1t")
        nc.sync.dma_start(b1t[:], b1_d.ap()[:])
        b2t = wpool.tile([128, H], F32, tag="b2t")
        nc.sync.dma_start(b2t[:], b2_d.ap()[:])

        # block-0 activations + the first w1 column chunks, interleaved per k
        # so the PE's first psum group can start as soon as possible. The
        # first column block of w1 is split into [128,128] tiles (w1s) so a
        # single slow queue can't hold back the first matmul group.
        bt = [[None] * K1 for _ in range(n_blocks)]
        w1s = [[None] * MB for _ in range(K1)]      # first col block, small
        w1 = [[None] * NMB for _ in range(K1)]      # remaining col blocks
        for k in range(K1):
            t = btpool.tile([128, R], F16, tag="bt")
            nc.sync.dma_start(t[:], bt_d.ap()[k * 128:(k + 1) * 128, 0:R])
            bt[0][k] = t
            for mo in range(MB):
                s = wpool.tile([128, 128], F16, tag=f"w1s{k}_{mo}")
                nc.sync.dma_start(
                    s[:], w1t_d.ap()[k * 128:(k + 1) * 128, mo * 128:(mo + 1) * 128]
                )
                w1s[k][mo] = s

        # w1 column blocks 1..3, then first two w2 h-groups, then w1 4..7,
        # then the rest of w2 — matches consumption order with slack.
        def emit_w1(mb):
            for k in range(K1):
                t = wpool.tile([128, MB * 128], F16, tag=f"w1_{k}_{mb}")
                nc.sync.dma_start(
                    t[:],
                    w1t_d.ap()[k * 128:(k + 1) * 128, mb * MB * 128:(mb + 1) * MB * 128],
                )
                w1[k][mb] = t

        w2 = [[None] * NMG for _ in range(H)]

        def emit_w2(h):
            for mg in range(NMG):
                t = wpool.tile([128, MG * 128], F16, tag=f"w2_{h}_{mg}")
                r0 = (h * NMG + mg) * 128
                nc.sync.dma_start(t[:], w2p_d.ap()[r0:r0 + 128, :])
                w2[h][mg] = t

        for mb in range(1, 4):
            emit_w1(mb)
        emit_w2(0)
        emit_w2(1)
        for mb in range(4, NMB):
            emit_w1(mb)
        for h in range(2, H):
            emit_w2(h)

        sig = wpool.tile([1, NC_ROWS], F32, tag="sig")

        accF = [None] * n_blocks       # final f32 dot accumulator per block

        def emit_tail(b):
            # partition-reduce 128 -> 1, sigmoid, and the block's output DMA
            psS = psumS.tile([1, R], F32)
            nc.tensor.matmul(psS[:], ones[:], accF[b][:], start=True, stop=True)
            nc.scalar.activation(
                sig[0:1, b * R:(b + 1) * R], psS[:],
                mybir.ActivationFunctionType.Sigmoid,
            )
            nc.sync.dma_start(out_d.ap()[0:1, b * R:(b + 1) * R],
                              sig[0:1, b * R:(b + 1) * R])

        for b in range(n_blocks):
            # prefetch next block's batchT (behind the weight bulk)
            if b + 1 < n_blocks:
                for k in range(K1):
                    t = btpool.tile([128, R], F16, tag="bt")
                    nc.sync.dma_start(
                        t[:],
                        bt_d.ap()[k * 128:(k + 1) * 128, (b + 1) * R:(b + 2) * R],
                    )
                    bt[b + 1][k] = t

            # ---- phase 1: innerT[m] = tanh(W1T.T @ batchT + b1) ----
            it = []
            for m in range(M1):
                mb, mo = divmod(m, MB)
                ps = psum1.tile([128, R], F32)
                for k in range(K1):
                    lhs = (w1s[k][mo][:] if mb == 0
                           else w1[k][mb][:, mo * 128:(mo + 1) * 128])
                    nc.tensor.matmul(
                        ps[:], lhs, bt[b][k][:],
                        start=(k == 0), stop=(k == K1 - 1),
                    )
                t = ipool.tile([128, R], F16, tag="it")
                nc.scalar.activation(
                    t[:], ps[:], mybir.ActivationFunctionType.Tanh,
                    bias=b1t[:, m:m + 1],
                )
                it.append(t)

            # deferred tail of the previous block: by now its DVE chain is
            # long done, so the reduce matmul costs PE no stall.
            if b > 0:
                emit_tail(b - 1)

            # ---- phase 2 + row-dot, per d_model chunk h ----
            acc = None
            for h in range(H):
                ps2 = psum2.tile([128, R], F32)
                for m in range(M1):
                    mg, j = divmod(m, MG)
                    nc.tensor.matmul(
                        ps2[:],
                        w2[h][mg][:, j * 128:(j + 1) * 128],
                        it[m][:],
                        start=(m == 0), stop=(m == M1 - 1),
                    )
                wx = vpool.tile([128, R], F16, tag="wx")
                nc.scalar.activation(
                    wx[:], ps2[:], mybir.ActivationFunctionType.Tanh,
                    bias=b2t[:, h:h + 1],
                )
                # dot contribution: acc += wxT_h * batchT_h   (f32 on DVE)
                if h == 0:
                    acc = vpool.tile([128, R], F32, tag="acc", bufs=4)
                    nc.vector.scalar_tensor_tensor(
                        out=acc[:], in0=wx[:], scalar=1.0, in1=bt[b][h][:],
                        op0=mybir.AluOpType.mult, op1=mybir.AluOpType.mult,
                    )
                else:
                    p = vpool.tile([128, R], F32, tag="p")
                    nc.vector.scalar_tensor_tensor(
                        out=p[:], in0=wx[:], scalar=1.0, in1=bt[b][h][:],
                        op0=mybir.AluOpType.mult, op1=mybir.AluOpType.mult,
                    )
                    nacc = vpool.tile([128, R], F32, tag="acc", bufs=4)
                    nc.vector.scalar_tensor_tensor(
                        out=nacc[:], in0=acc[:], scalar=1.0, in1=p[:],
                        op0=mybir.AluOpType.mult, op1=mybir.AluOpType.add,
                    )
                    acc = nacc
            accF[b] = acc

        emit_tail(n_blocks - 1)

    return nc


_CACHED = {}


def _get_nc(n_blocks=N_BLOCKS):
    if n_blocks not in _CACHED:
        _CACHED[n_blocks] = build_bass(n_blocks)
    return _CACHED[n_blocks]


def _prep_in_maps(batch, W1, b1, W2, b2):
    batch = np.ascontiguousarray(batch, dtype=np.float32)
    w1t = np.ascontiguousarray(W1.T, dtype=np.float16)      # [1024, 4096]
    w2t = np.asarray(W2, dtype=np.float16).T                # [4096, 1024]
    # pack so tile (h, mg) rows (h*NMG+mg)*128.. hold cols j*128+c =
    # W2T[(mg*MG+j)*128+p, h*128+c]
    w2p = np.ascontiguousarray(
        w2t.reshape(NMG, MG, 128, H, 128).transpose(3, 0, 2, 1, 4)
        .reshape(H * NMG * 128, MG * 128)
    )
    # b1 as [128, 32]: column m holds b1[m*128:(m+1)*128] (per-partition bias)
    b1c = np.ascontiguousarray(np.asarray(b1, dtype=np.float32).reshape(M1, 128).T)
    b2c = np.ascontiguousarray(np.asarray(b2, dtype=np.float32).reshape(H, 128).T)
    ones = np.ones((128, 1), dtype=np.float32)
    batcht = np.ascontiguousarray(batch.T.astype(np.float16))  # [1024, 16384]

    in_maps = []
    for c in range(N_CORES):
        r0, r1 = c * NC_ROWS, (c + 1) * NC_ROWS
        in_maps.append({
            "w1t": w1t,
            "w2p": w2p,
            "b1c": b1c,
            "b2c": b2c,
            "ones": ones,
            "batcht": np.ascontiguousarray(batcht[:, r0:r1]),
        })
    return in_maps


def kernel(batch, W1, b1, W2, b2, _trace=False, _trace_kwargs=None):
    in_maps = _prep_in_maps(batch, W1, b1, W2, b2)
    nc = _get_nc()
    res = bass_utils.run_bass_kernel_spmd(
        nc, in_maps, core_ids=list(range(N_CORES)),
        trace=_trace, **(_trace_kwargs or {}),
    )
    out = np.concatenate([res.results[c]["out"][0] for c in range(N_CORES)])
    if _trace:
        return out, res
    return out


# revision 10
# speedup vs baseline: 1.0617x; 1.0344x over previous
"""Trainium2 kernel for nn_MiddleHeadLayer: 2-layer tanh MLP + row-dot + sigmoid.

    inner = tanh(batch @ W1.T + b1)        batch [N, 1024], W1 [4096, 1024]
    wx    = tanh(inner @ W2.T + b2)        W2 [1024, 4096]
    out   = sigmoid(sum(wx * batch, -1))   [N]

Data-parallel over 8 NeuronCores: each core handles N/8 = 2048 rows;
weights replicated, resident in SBUF as fp16 (f32 weights do not fit in
24MB SBUF; fp16 matmuls run at full PE rate and keep absmax error ~3e-3).

Per-core dataflow, in blocks of R=512 rows:
  phase 1: innerT[dff, rows] = tanh(W1T.T @ batchT + b1) — stationary W1T
           chunks [128,128], moving batchT [128, 512], fp16 in / f32 PSUM,
           ACT applies the per-partition (d_ff) bias and writes fp16.
  phase 2: wxT[dmodel, rows] = tanh(W2 @ inner.T + b2) — stationary W2T
           chunks [128 dff, 128 dmodel], moving innerT [128, 512]. The
           output partition dim is d_model, so b2 is a plain per-partition
           ACT bias (no extra rank-1 bias matmul needed).
  dot:     acc[dm_chunk, rows] = sum_h wxT_h * batchT_h on DVE (the same
           fp16 batchT tiles phase 1 streams), then a [128,1] ones fp32
           matmul folds the 128 partitions into s[1, rows]; sigmoid on ACT;
           one contiguous 2KB output DMA per block.

DMA order is chosen so the first matmul can start ~4us in: batchT block 0
and the first W1T column chunks go first in small pieces, then W1/W2
stream in exactly the order the PE consumes them.
"""

from contextlib import ExitStack

import numpy as np
import orjson

import concourse.bass as bass
import concourse.tile as tile
from concourse import mybir
from concourse import bass_utils

D_MODEL = 1024
D_FF = 4096
N_TOTAL = 16384
N_CORES = 8
NC_ROWS = N_TOTAL // N_CORES          # 2048 rows per core
R = 512                               # row-block size
N_BLOCKS = NC_ROWS // R               # 4
K1 = D_MODEL // 128                   # 8 contraction chunks for matmul1
M1 = D_FF // 128                      # 32 d_ff chunks
MB = 4                                # m-chunks per w1 column tile
NMB = M1 // MB                        # 8 w1 column tiles per k
H = D_MODEL // 128                    # 8 d_model chunks in phase 2
MG = 4                                # m-chunks per w2 tile
NMG = M1 // MG                        # 8 w2 tiles per h
F16 = mybir.dt.float16
F32 = mybir.dt.float32


# ---------------------------------------------------------------------------
# This walrus build rejects >2 sem waits on a single instruction, while Tile's
# wait assignment freely attaches more (e.g. the exit drain gets one wait per
# outstanding logical proc). Legalize at the BIR-JSON level: hoist excess
# waits onto EventSemaphore instructions inserted directly before the
# offending instruction on the same engine stream (identical semantics).
MAX_WAITS = 1
ESEM_WAITS = 2  # EventSemaphore instructions themselves may carry 2 waits


def _legalize_sync_waits(bir: dict) -> dict:
    ctr = 0
    for fn in bir.get("functions", []):
        for blk in fn.get("blocks", []):
            insts = blk.get("instructions")
            if not insts:
                continue
            out = []
            changed = False
            for inst in insts:
                si = inst.get("sync_info")
                ow = (si or {}).get("on_wait") or []
                limit = ESEM_WAITS if inst.get("opcode") == "EventSemaphore" else MAX_WAITS
                if len(ow) > limit:
                    changed = True
                    excess, keep = ow[:-limit], ow[-limit:]
                    for i in range(0, len(excess), ESEM_WAITS):
                        ctr += 1
                        out.append({
                            "debug": inst.get("debug"),
                            "engine": inst["engine"],
                            "ins": [],
                            "outs": [],
                            "name": f"legalwait-{ctr}",
                            "opcode": "EventSemaphore",
                            "sync_info": {
                                "on_update": [],
                                "on_wait": excess[i:i + ESEM_WAITS],
                            },
                        })
                    si["on_wait"] = keep
                out.append(inst)
            if changed:
                blk["instructions"] = out
    return bir


_orig_to_json_bytes = bass.Bass.to_json_bytes


def _patched_to_json_bytes(self) -> bytes:
    return orjson.dumps(_legalize_sync_waits(orjson.loads(_orig_to_json_bytes(self))))


bass.Bass.to_json_bytes = _patched_to_json_bytes


def build_bass(n_blocks=N_BLOCKS):
    nc = bass.Bass("TRN2", target_bir_lowering=False, debug=False)

    w1t_d = nc.dram_tensor("w1t", [D_MODEL, D_FF], F16, kind="ExternalInput")
    # w2p: W2.T pre-packed host-side as [H, NMG, 128, MG*128] so each
    # (h, mg) stationary group is one contiguous 128KB tile.
    w2p_d = nc.dram_tensor("w2p", [H * NMG * 128, MG * 128], F16, kind="ExternalInput")
    b1_d = nc.dram_tensor("b1c", [128, M1], F32, kind="ExternalInput")
    b2_d = nc.dram_tensor("b2c", [128, H], F32, kind="ExternalInput")
    ones_d = nc.dram_tensor("ones", [128, 1], F16, kind="ExternalInput")
    bt_d = nc.dram_tensor("batcht", [D_MODEL, NC_ROWS], F16, kind="ExternalInput")
    out_d = nc.dram_tensor("out", [1, NC_ROWS], F32, kind="ExternalOutput")

    N_WARM = 48                        # HAM warm-up matmuls during DMA wait

    with tile.TileContext(nc) as tc, ExitStack() as ctx:
        wpool = ctx.enter_context(tc.tile_pool(name="weights", bufs=1))
        btpool = ctx.enter_context(tc.tile_pool(name="batchT", bufs=16))
        ipool = ctx.enter_context(tc.tile_pool(name="innerT", bufs=32))
        vpool = ctx.enter_context(tc.tile_pool(name="dot", bufs=2))
        psum1 = ctx.enter_context(tc.tile_pool(name="psum1", bufs=2, space="PSUM"))
        psum2 = ctx.enter_context(tc.tile_pool(name="psum2", bufs=2, space="PSUM"))
        psumS = ctx.enter_context(tc.tile_pool(name="psumS", bufs=1, space="PSUM"))
        psumW = ctx.enter_context(tc.tile_pool(name="psumW", bufs=1, space="PSUM"))

        # --- DMA emission order = queue order -----------------------------
        # A tiny warm tile goes first; a burst of junk matmuls on it keeps
        # the PE busy from ~3us so the HAM clock gate opens (1.2 -> 2.4 GHz)
        # before the real, DMA-gated first block starts.
        warm = wpool.tile([128, 64], F16, tag="warm")
        nc.sync.dma_start(warm[:], w1t_d.ap()[0:128, 0:64])
        psw = psumW.tile([64, 64], F32)
        for _ in range(N_WARM):
            nc.tensor.matmul(psw[:], warm[:], warm[:], start=True, stop=True)

        # block-0 activations + the first w1 column block, interleaved per k
        # so the PE's first psum group can start as soon as possible.
        bt = [[None] * K1 for _ in range(n_blocks)]
        w1 = [[None] * NMB for _ in range(K1)]

        def emit_w1(mb):
            for k in range(K1):
                t = wpool.tile([128, MB * 128], F16, tag=f"w1_{k}_{mb}")
                nc.sync.dma_start(
                    t[:],
                    w1t_d.ap()[k * 128:(k + 1) * 128, mb * MB * 128:(mb + 1) * MB * 128],
                )
                w1[k][mb] = t

        for k in range(K1):
            t = btpool.tile([128, R], F16, tag="bt")
            nc.sync.dma_start(t[:], bt_d.ap()[k * 128:(k + 1) * 128, 0:R])
            bt[0][k] = t
            tw = wpool.tile([128, MB * 128], F16, tag=f"w1_{k}_0")
            nc.sync.dma_start(tw[:], w1t_d.ap()[k * 128:(k + 1) * 128, 0:MB * 128])
            w1[k][0] = tw

        ones = wpool.tile([128, 1], F16, tag="ones")
        nc.sync.dma_start(ones[:], ones_d.ap()[:])
        b1t = wpool.tile([128, M1], F32, tag="b1t")
        nc.sync.dma_start(b1t[:], b1_d.ap()[:])
        b2t = wpool.tile([128, H], F32, tag="b2t")
        nc.sync.dma_start(b2t[:], b2_d.ap()[:])

        # remaining w1 column blocks and w2 h-groups, in consumption order
        # with slack.
        w2 = [[None] * NMG for _ in range(H)]

        def emit_w2(h):
            for mg in range(NMG):
                t = wpool.tile([128, MG * 128], F16, tag=f"w2_{h}_{mg}")
                r0 = (h * NMG + mg) * 128
                nc.sync.dma_start(t[:], w2p_d.ap()[r0:r0 + 128, :])
                w2[h][mg] = t

        for mb in range(1, 4):
            emit_w1(mb)
        emit_w2(0)
        for mb in range(4, 6):
            emit_w1(mb)
        emit_w2(1)
        for mb in range(6, NMB):
            emit_w1(mb)
        for h in range(2, H):
            emit_w2(h)

        sig = wpool.tile([1, NC_ROWS], F32, tag="sig")

        accF = [None] * n_blocks       # final f32 dot accumulator per block

        def emit_tail(b):
            # partition-reduce 128 -> 1, sigmoid, and the block's output DMA.
            # accF is fp16 so the reduce matmul runs in one pass (fp32 moving
            # operands cost two half-speed passes on the PE).
            psS = psumS.tile([1, R], F32)
            if isinstance(accF[b], tuple):          # last block: split halves
                a0, a1 = accF[b]
                nc.tensor.matmul(psS[:, 0:R // 2], ones[:], a0[:], start=True,
                                 stop=True)
                nc.tensor.matmul(psS[:, R // 2:R], ones[:], a1[:], start=True,
                                 stop=True)
            else:
                nc.tensor.matmul(psS[:], ones[:], accF[b][:], start=True, stop=True)
            nc.scalar.activation(
                sig[0:1, b * R:(b + 1) * R], psS[:],
                mybir.ActivationFunctionType.Sigmoid,
            )
            nc.sync.dma_start(out_d.ap()[0:1, b * R:(b + 1) * R],
                              sig[0:1, b * R:(b + 1) * R])

        for b in range(n_blocks):
            # prefetch next block's batchT (behind the weight bulk)
            if b + 1 < n_blocks:
                for k in range(K1):
                    t = btpool.tile([128, R], F16, tag="bt")
                    nc.sync.dma_start(
                        t[:],
                        bt_d.ap()[k * 128:(k + 1) * 128, (b + 1) * R:(b + 2) * R],
                    )
                    bt[b + 1][k] = t

            # ---- phase 1: innerT[m] = tanh(W1T.T @ batchT + b1) ----
            it = []
            for m in range(M1):
                mb, mo = divmod(m, MB)
                ps = psum1.tile([128, R], F32)
                for k in range(K1):
                    nc.tensor.matmul(
                        ps[:], w1[k][mb][:, mo * 128:(mo + 1) * 128], bt[b][k][:],
                        start=(k == 0), stop=(k == K1 - 1),
                    )
                t = ipool.tile([128, R], F16, tag="it")
                nc.scalar.activation(
                    t[:], ps[:], mybir.ActivationFunctionType.Tanh,
                    bias=b1t[:, m:m + 1],
                )
                it.append(t)

            # deferred tail of the previous block: by now its DVE chain is
            # long done, so the reduce matmul costs PE no stall.
            if b > 0:
                emit_tail(b - 1)

            # ---- phase 2 + row-dot, per d_model chunk h ----
            # The dot (acc += wxT_h * batchT_h, f32 on DVE) trails each h
            # chunk; the final add converts to fp16 so the partition-reduce
            # matmul in emit_tail runs at full PE rate.
            last_blk = b == n_blocks - 1

            def dot_step(h, wx, acc, cols, final):
                c0, c1 = cols
                if acc is None:
                    out = vpool.tile([128, c1 - c0], F32, tag="acc", bufs=4,
                                     name="acc0")
                    nc.vector.scalar_tensor_tensor(
                        out=out[:], in0=wx[:], scalar=1.0,
                        in1=bt[b][h][:, c0:c1],
                        op0=mybir.AluOpType.mult, op1=mybir.AluOpType.mult,
                    )
                    return out
                p = vpool.tile([128, c1 - c0], F32, tag="p", name="p")
                nc.vector.scalar_tensor_tensor(
                    out=p[:], in0=wx[:], scalar=1.0, in1=bt[b][h][:, c0:c1],
                    op0=mybir.AluOpType.mult, op1=mybir.AluOpType.mult,
                )
                if final:
                    out = vpool.tile([128, c1 - c0], F16, tag="acc16", bufs=2,
                                     name="acc16")
                else:
                    out = vpool.tile([128, c1 - c0], F32, tag="acc", bufs=4,
                                     name="accn")
                nc.vector.scalar_tensor_tensor(
                    out=out[:], in0=acc[:], scalar=1.0, in1=p[:],
                    op0=mybir.AluOpType.mult, op1=mybir.AluOpType.add,
                )
                return out

            acc = None
            for h in range(H):
                split = last_blk and h == H - 1
                if not split:
                    ps2 = psum2.tile([128, R], F32)
                    for m in range(M1):
                        mg, j = divmod(m, MG)
                        nc.tensor.matmul(
                            ps2[:], w2[h][mg][:, j * 128:(j + 1) * 128], it[m][:],
                            start=(m == 0), stop=(m == M1 - 1),
                        )
                    wx = vpool.tile([128, R], F16, tag="wx")
                    nc.scalar.activation(
                        wx[:], ps2[:], mybir.ActivationFunctionType.Tanh,
                        bias=b2t[:, h:h + 1],
                    )
                    acc = dot_step(h, wx, acc, (0, R), final=(h == H - 1))
                else:
                    # last h of the last block in two column halves so most
                    # of the ACT/DVE/reduce tail overlaps the second half's
                    # matmuls instead of trailing the whole kernel.
                    halves = []
                    for half in range(2):
                        c0, c1 = half * R // 2, (half + 1) * R // 2
                        psh = psum2.tile([128, R // 2], F32,
                                         tag=f"h7{half}", bufs=1)
                        for m in range(M1):
                            mg, j = divmod(m, MG)
                            nc.tensor.matmul(
                                psh[:], w2[h][mg][:, j * 128:(j + 1) * 128],
                                it[m][:, c0:c1],
                                start=(m == 0), stop=(m == M1 - 1),
                            )
                        wxh = vpool.tile([128, R // 2], F16, tag="wxh")
                        nc.scalar.activation(
                            wxh[:], psh[:], mybir.ActivationFunctionType.Tanh,
                            bias=b2t[:, h:h + 1],
                        )
                        ph = vpool.tile([128, R // 2], F32, tag="ph", name="ph")
                        nc.vector.scalar_tensor_tensor(
                            out=ph[:], in0=wxh[:], scalar=1.0,
                            in1=bt[b][h][:, c0:c1],
                            op0=mybir.AluOpType.mult, op1=mybir.AluOpType.mult,
                        )
                        a16 = vpool.tile([128, R // 2], F16, tag="acc16h",
                                         name="a16")
                        nc.vector.scalar_tensor_tensor(
                            out=a16[:], in0=acc[:, c0:c1], scalar=1.0, in1=ph[:],
                            op0=mybir.AluOpType.mult, op1=mybir.AluOpType.add,
                        )
                        halves.append(a16)
                    acc = tuple(halves)
            accF[b] = acc

        emit_tail(n_blocks - 1)

    return nc


_CACHED = {}


def _get_nc(n_blocks=N_BLOCKS):
    if n_blocks not in _CACHED:
        _CACHED[n_blocks] = build_bass(n_blocks)
    return _CACHED[n_blocks]


def _prep_in_maps(batch, W1, b1, W2, b2):
    batch = np.ascontiguousarray(batch, dtype=np.float32)
    w1t = np.ascontiguousarray(W1.T, dtype=np.float16)      # [1024, 4096]
    w2t = np.asarray(W2, dtype=np.float16).T                # [4096, 1024]
    # pack so tile (h, mg) rows (h*NMG+mg)*128.. hold cols j*128+c =
    # W2T[(mg*MG+j)*128+p, h*128+c]
    w2p = np.ascontiguousarray(
        w2t.reshape(NMG, MG, 128, H, 128).transpose(3, 0, 2, 1, 4)
        .reshape(H * NMG * 128, MG * 128)
    )
    # b1 as [128, 32]: column m holds b1[m*128:(m+1)*128] (per-partition bias)
    b1c = np.ascontiguousarray(np.asarray(b1, dtype=np.float32).reshape(M1, 128).T)
    b2c = np.ascontiguousarray(np.asarray(b2, dtype=np.float32).reshape(H, 128).T)
    ones = np.ones((128, 1), dtype=np.float16)
    batcht = np.ascontiguousarray(batch.T.astype(np.float16))  # [1024, 16384]

    in_maps = []
    for c in range(N_CORES):
        r0, r1 = c * NC_ROWS, (c + 1) * NC_ROWS
        in_maps.append({
            "w1t": w1t,
            "w2p": w2p,
            "b1c": b1c,
            "b2c": b2c,
            "ones": ones,
            "batcht": np.ascontiguousarray(batcht[:, r0:r1]),
        })
    return in_maps


def kernel(batch, W1, b1, W2, b2, _trace=False, _trace_kwargs=None):
    in_maps = _prep_in_maps(batch, W1, b1, W2, b2)
    nc = _get_nc()
    res = bass_utils.run_bass_kernel_spmd(
        nc, in_maps, core_ids=list(range(N_CORES)),
        trace=_trace, **(_trace_kwargs or {}),
    )
    out = np.concatenate([res.results[c]["out"][0] for c in range(N_CORES)])
    if _trace:
        return out, res
    return out


# revision 12
# speedup vs baseline: 1.0860x; 1.0229x over previous
"""Trainium2 kernel for nn_MiddleHeadLayer: 2-layer tanh MLP + row-dot + sigmoid.

    inner = tanh(batch @ W1.T + b1)        batch [N, 1024], W1 [4096, 1024]
    wx    = tanh(inner @ W2.T + b2)        W2 [1024, 4096]
    out   = sigmoid(sum(wx * batch, -1))   [N]

Data-parallel over 8 NeuronCores: each core handles N/8 = 2048 rows;
weights replicated, resident in SBUF as fp16 (fp16 matmuls run at full PE
rate, 1 moving column/cycle; absmax error stays ~4e-3).

Per-core dataflow, in blocks of R=512 rows:
  phase 1: innerT[dff, rows] = tanh(W1T.T @ batchT + b1) — stationary W1T
           chunks [128,128], moving batchT [128, 512], fp16 in / f32 PSUM,
           ACT applies the per-partition (d_ff) bias and writes fp16.
  phase 2: wxT[dmodel, rows] = tanh(W2 @ inner.T + b2) — stationary W2T
           chunks [128 dff, 128 dmodel], moving innerT [128, 512]. Output
           partitions are d_model, so b2 is a per-partition ACT bias.
  dot:     acc[dm_chunk, rows] = sum_h wxT_h * batchT_h on DVE (reusing the
           fp16 batchT tiles phase 1 streams); final add writes fp16 so the
           [128,1]-ones partition-reduce matmul runs in one PE pass;
           sigmoid on ACT; one contiguous 2KB output DMA per block.

DMA strategy: the Sync sequencer issues one DMA every ~580ns, so DMA
COUNT (not just bytes) is the startup constraint. All weights and
activations are host-packed so each logical group is ONE contiguous
[128, 4096] 1MB DMA (~78% DMA efficiency): 8 for W1, 8 for W2, 4 for
batchT. The 128-partition slab meaning of each 512-column span differs
(it encodes the contraction chunk), which the matmul APs slice out.
A memset-fed burst of junk matmuls warms the PE clock gate (HAM,
1.2 -> 2.4 GHz) while the first DMAs are in flight.
"""

from contextlib import ExitStack

import numpy as np
import orjson

import concourse.bass as bass
import concourse.tile as tile
from concourse import mybir
from concourse import bass_utils

D_MODEL = 1024
D_FF = 4096
N_TOTAL = 16384
N_CORES = 8
NC_ROWS = N_TOTAL // N_CORES          # 2048 rows per core
R = 512                               # row-block size
N_BLOCKS = NC_ROWS // R               # 4
K1 = D_MODEL // 128                   # 8 contraction chunks for matmul1
M1 = D_FF // 128                      # 32 d_ff chunks
MB = 4                                # m-chunks per w1 column block
NMB = M1 // MB                        # 8 w1 column blocks
H = D_MODEL // 128                    # 8 d_model chunks in phase 2
F16 = mybir.dt.float16
F32 = mybir.dt.float32
N_WARM = 170                          # HAM warm-up matmuls during DMA wait


# ---------------------------------------------------------------------------
# This walrus build rejects >2 sem waits on a single instruction, while Tile's
# wait assignment freely attaches more (e.g. the exit drain gets one wait per
# outstanding logical proc). Legalize at the BIR-JSON level: hoist excess
# waits onto EventSemaphore instructions inserted directly before the
# offending instruction on the same engine stream (identical semantics).
MAX_WAITS = 1
ESEM_WAITS = 2  # EventSemaphore instructions themselves may carry 2 waits


def _legalize_sync_waits(bir: dict) -> dict:
    ctr = 0
    for fn in bir.get("functions", []):
        for blk in fn.get("blocks", []):
            insts = blk.get("instructions")
            if not insts:
                continue
            out = []
            changed = False
            for inst in insts:
                si = inst.get("sync_info")
                ow = (si or {}).get("on_wait") or []
                limit = ESEM_WAITS if inst.get("opcode") == "EventSemaphore" else MAX_WAITS
                if len(ow) > limit:
                    changed = True
                    excess, keep = ow[:-limit], ow[-limit:]
                    for i in range(0, len(excess), ESEM_WAITS):
                        ctr += 1
                        out.append({
                            "debug": inst.get("debug"),
                            "engine": inst["engine"],
                            "ins": [],
                            "outs": [],
                            "name": f"legalwait-{ctr}",
                            "opcode": "EventSemaphore",
                            "sync_info": {
                                "on_update": [],
                                "on_wait": excess[i:i + ESEM_WAITS],
                            },
                        })
                    si["on_wait"] = keep
                out.append(inst)
            if changed:
                blk["instructions"] = out
    return bir


_orig_to_json_bytes = bass.Bass.to_json_bytes


def _patched_to_json_bytes(self) -> bytes:
    return orjson.dumps(_legalize_sync_waits(orjson.loads(_orig_to_json_bytes(self))))


bass.Bass.to_json_bytes = _patched_to_json_bytes


def build_bass(n_blocks=N_BLOCKS):
    nc = bass.Bass("TRN2", target_bir_lowering=False, debug=False)

    # w1p row-block mb: [128, 4096], cols k*512 + mo*128 + c hold
    #   W1T[k*128+p, mb*512 + mo*128 + c]
    w1p_d = nc.dram_tensor("w1p", [NMB * 128, K1 * 512], F16, kind="ExternalInput")
    # w2p row-block h: [128, 4096], cols m*128 + c hold W2T[m*128+p, h*128+c]
    w2p_d = nc.dram_tensor("w2p", [H * 128, M1 * 128], F16, kind="ExternalInput")
    b1_d = nc.dram_tensor("b1c", [128, M1], F32, kind="ExternalInput")
    b2_d = nc.dram_tensor("b2c", [128, H], F32, kind="ExternalInput")
    ones_d = nc.dram_tensor("ones", [128, 1], F16, kind="ExternalInput")
    # btp row-block b: [128, 4096], cols k*512 + r hold batch[b*512+r, k*128+p]
    btp_d = nc.dram_tensor("btp", [N_BLOCKS * 128, K1 * R], F16, kind="ExternalInput")
    out_d = nc.dram_tensor("out", [1, NC_ROWS], F32, kind="ExternalOutput")

    with tile.TileContext(nc) as tc, ExitStack() as ctx:
        wpool = ctx.enter_context(tc.tile_pool(name="weights", bufs=1))
        btpool = ctx.enter_context(tc.tile_pool(name="batchT", bufs=2))
        ipool = ctx.enter_context(tc.tile_pool(name="innerT", bufs=32))
        vpool = ctx.enter_context(tc.tile_pool(name="dot", bufs=2))
        psum1 = ctx.enter_context(tc.tile_pool(name="psum1", bufs=2, space="PSUM"))
        psum2 = ctx.enter_context(tc.tile_pool(name="psum2", bufs=2, space="PSUM"))
        psumS = ctx.enter_context(tc.tile_pool(name="psumS", bufs=1, space="PSUM"))
        psumW = ctx.enter_context(tc.tile_pool(name="psumW", bufs=1, space="PSUM"))

        # --- HAM warm-up: junk matmuls on a memset tile (no DMA dep) ------
        warm = wpool.tile([128, 64], F16, tag="warm")
        nc.vector.memset(warm[:], 0.001953125)
        psw = psumW.tile([64, 64], F32)
        for _ in range(N_WARM):
            nc.tensor.matmul(psw[:], warm[:], warm[:], start=True, stop=True)

        # --- DMA emission order = consumption order -----------------------
        bt = [None] * n_blocks
        bt[0] = btpool.tile([128, K1 * R], F16, tag="bt", name="bt0")
        nc.sync.dma_start(bt[0][:], btp_d.ap()[0:128, :])

        w1 = [None] * NMB

        def emit_w1(mb):
            t = wpool.tile([128, K1 * 512], F16, tag=f"w1_{mb}")
            nc.sync.dma_start(t[:], w1p_d.ap()[mb * 128:(mb + 1) * 128, :])
            w1[mb] = t

        emit_w1(0)

        ones = wpool.tile([128, 1], F16, tag="ones")
        nc.sync.dma_start(ones[:], ones_d.ap()[:])
        b1t = wpool.tile([128, M1], F32, tag="b1t")
        nc.sync.dma_start(b1t[:], b1_d.ap()[:])
        b2t = wpool.tile([128, H], F32, tag="b2t")
        nc.sync.dma_start(b2t[:], b2_d.ap()[:])

        w2 = [None] * H

        def emit_w2(h):
            t = wpool.tile([128, M1 * 128], F16, tag=f"w2_{h}")
            nc.sync.dma_start(t[:], w2p_d.ap()[h * 128:(h + 1) * 128, :])
            w2[h] = t

        for mb in range(1, 4):
            emit_w1(mb)
        emit_w2(0)
        for mb in range(4, 6):
            emit_w1(mb)
        emit_w2(1)
        for mb in range(6, NMB):
            emit_w1(mb)
        for h in range(2, H):
            emit_w2(h)

        sig = wpool.tile([1, NC_ROWS], F32, tag="sig")

        accF = [None] * n_blocks       # final fp16 dot accumulator per block

        def emit_tail(b):
            # partition-reduce 128 -> 1, sigmoid, and the block's output DMA
            psS = psumS.tile([1, R], F32)
            if isinstance(accF[b], tuple):          # last block: split halves
                a0, a1 = accF[b]
                nc.tensor.matmul(psS[:, 0:R // 2], ones[:], a0[:], start=True,
                                 stop=True)
                nc.tensor.matmul(psS[:, R // 2:R], ones[:], a1[:], start=True,
                                 stop=True)
            else:
                nc.tensor.matmul(psS[:], ones[:], accF[b][:], start=True, stop=True)
            nc.scalar.activation(
                sig[0:1, b * R:(b + 1) * R], psS[:],
                mybir.ActivationFunctionType.Sigmoid,
            )
            nc.sync.dma_start(out_d.ap()[0:1, b * R:(b + 1) * R],
                              sig[0:1, b * R:(b + 1) * R])

        for b in range(n_blocks):
            # prefetch next block's batchT (queued behind the weight bulk)
            if b + 1 < n_blocks:
                t = btpool.tile([128, K1 * R], F16, tag="bt")
                nc.sync.dma_start(t[:], btp_d.ap()[(b + 1) * 128:(b + 2) * 128, :])
                bt[b + 1] = t

            # ---- phase 1: innerT[m] = tanh(W1T.T @ batchT + b1) ----
            it = []
            for m in range(M1):
                mb, mo = divmod(m, MB)
                ps = psum1.tile([128, R], F32)
                for k in range(K1):
                    nc.tensor.matmul(
                        ps[:],
                        w1[mb][:, k * 512 + mo * 128:k * 512 + (mo + 1) * 128],
                        bt[b][:, k * R:(k + 1) * R],
                        start=(k == 0), stop=(k == K1 - 1),
                    )
                t = ipool.tile([128, R], F16, tag="it")
                nc.scalar.activation(
                    t[:], ps[:], mybir.ActivationFunctionType.Tanh,
                    bias=b1t[:, m:m + 1],
                )
                it.append(t)

            # deferred tail of the previous block: by now its DVE chain is
            # long done, so the reduce matmul costs PE no stall.
            if b > 0:
                emit_tail(b - 1)

            # ---- phase 2 + row-dot, per d_model chunk h ----
            last_blk = b == n_blocks - 1
            acc = None
            for h in range(H):
                if not (last_blk and h == H - 1):
                    ps2 = psum2.tile([128, R], F32)
                    for m in range(M1):
                        nc.tensor.matmul(
                            ps2[:], w2[h][:, m * 128:(m + 1) * 128], it[m][:],
                            start=(m == 0), stop=(m == M1 - 1),
                        )
                    wx = vpool.tile([128, R], F16, tag="wx")
                    nc.scalar.activation(
                        wx[:], ps2[:], mybir.ActivationFunctionType.Tanh,
                        bias=b2t[:, h:h + 1],
                    )
                    final = h == H - 1
                    if h == 0:
                        acc = vpool.tile([128, R], F32, tag="acc", bufs=4,
                                         name="acc0")
                        nc.vector.scalar_tensor_tensor(
                            out=acc[:], in0=wx[:], scalar=1.0,
                            in1=bt[b][:, h * R:(h + 1) * R],
                            op0=mybir.AluOpType.mult, op1=mybir.AluOpType.mult,
                        )
                    else:
                        p = vpool.tile([128, R], F32, tag="p", name="p")
                        nc.vector.scalar_tensor_tensor(
                            out=p[:], in0=wx[:], scalar=1.0,
                            in1=bt[b][:, h * R:(h + 1) * R],
                            op0=mybir.AluOpType.mult, op1=mybir.AluOpType.mult,
                        )
                        nacc = vpool.tile(
                            [128, R], F16 if final else F32,
                            tag="acc16" if final else "acc",
                            bufs=2 if final else 4, name="accn",
                        )
                        nc.vector.scalar_tensor_tensor(
                            out=nacc[:], in0=acc[:], scalar=1.0, in1=p[:],
                            op0=mybir.AluOpType.mult, op1=mybir.AluOpType.add,
                        )
                        acc = nacc
                else:
                    # last h of the last block in two column halves so most
                    # of the ACT/DVE/reduce tail overlaps the second half's
                    # matmuls instead of trailing the whole kernel.
                    halves = []
                    for half in range(2):
                        c0, c1 = half * R // 2, (half + 1) * R // 2
                        psh = psum2.tile([128, R // 2], F32,
                                         tag=f"h7{half}", bufs=1)
                        for m in range(M1):
                            nc.tensor.matmul(
                                psh[:], w2[h][:, m * 128:(m + 1) * 128],
                                it[m][:, c0:c1],
                                start=(m == 0), stop=(m == M1 - 1),
                            )
                        wxh = vpool.tile([128, R // 2], F16, tag="wxh")
                        nc.scalar.activation(
                            wxh[:], psh[:], mybir.ActivationFunctionType.Tanh,
                            bias=b2t[:, h:h + 1],
                        )
                        ph = vpool.tile([128, R // 2], F32, tag="ph", name="ph")
                        nc.vector.scalar_tensor_tensor(
                            out=ph[:], in0=wxh[:], scalar=1.0,
                            in1=bt[b][:, h * R + c0:h * R + c1],
                            op0=mybir.AluOpType.mult, op1=mybir.AluOpType.mult,
                        )
                        a16 = vpool.tile([128, R // 2], F16, tag="acc16h",
                                         name="a16")
                        nc.vector.scalar_tensor_tensor(
                            out=a16[:], in0=acc[:, c0:c1], scalar=1.0, in1=ph[:],
                            op0=mybir.AluOpType.mult, op1=mybir.AluOpType.add,
                        )
                        halves.append(a16)
                    acc = tuple(halves)
            accF[b] = acc

        emit_tail(n_blocks - 1)

    return nc


_CACHED = {}


def _get_nc(n_blocks=N_BLOCKS):
    if n_blocks not in _CACHED:
        _CACHED[n_blocks] = build_bass(n_blocks)
    return _CACHED[n_blocks]


def _prep_in_maps(batch, W1, b1, W2, b2):
    batch = np.ascontiguousarray(batch, dtype=np.float32)
    w1t = np.asarray(W1, dtype=np.float16).T                # [1024, 4096]
    # [k, p, mb, cc] -> [mb, p, k, cc]
    w1p = np.ascontiguousarray(
        w1t.reshape(K1, 128, NMB, 512).transpose(2, 1, 0, 3)
        .reshape(NMB * 128, K1 * 512)
    )
    w2t = np.asarray(W2, dtype=np.float16).T                # [4096, 1024]
    # [m, p, h, c] -> [h, p, m, c]
    w2p = np.ascontiguousarray(
        w2t.reshape(M1, 128, H, 128).transpose(2, 1, 0, 3)
        .reshape(H * 128, M1 * 128)
    )
    b1c = np.ascontiguousarray(np.asarray(b1, dtype=np.float32).reshape(M1, 128).T)
    b2c = np.ascontiguousarray(np.asarray(b2, dtype=np.float32).reshape(H, 128).T)
    ones = np.ones((128, 1), dtype=np.float16)
    batcht = np.ascontiguousarray(batch.T.astype(np.float16))  # [1024, 16384]

    in_maps = []
    for c in range(N_CORES):
        r0, r1 = c * NC_ROWS, (c + 1) * NC_ROWS
        # [k, p, b, r] -> [b, p, k, r]
        btp = np.ascontiguousarray(
            batcht[:, r0:r1].reshape(K1, 128, N_BLOCKS, R).transpose(2, 1, 0, 3)
            .reshape(N_BLOCKS * 128, K1 * R)
        )
        in_maps.append({
            "w1p": w1p,
            "w2p": w2p,
            "b1c": b1c,
            "b2c": b2c,
            "ones": ones,
            "btp": btp,
        })
    return in_maps


def kernel(batch, W1, b1, W2, b2, _trace=False, _trace_kwargs=None):
    in_maps = _prep_in_maps(batch, W1, b1, W2, b2)
    nc = _get_nc()
    res = bass_utils.run_bass_kernel_spmd(
        nc, in_maps, core_ids=list(range(N_CORES)),
        trace=_trace, **(_trace_kwargs or {}),
    )
    out = np.concatenate([res.results[c]["out"][0] for c in range(N_CORES)])
    if _trace:
        return out, res
    return out
